# revision 46
# baseline (speedup 1.0000x reference)
"""Trainium2 Bass kernel for a single-head causal attention block.

Head-split (tensor-parallel) sharding over 8 NeuronCores: core = 2*b + half.
Each core handles one batch and HALF of the head dimension (1024 of 2048)
for ALL T queries/keys:

    per core:  q,k,v = x @ W*[:, half] + b*[half]          [T, H/2]
               partial[t, s] = (q @ k.T) * C**-0.5         [T, T] causal
               AllReduce(partial, pair) -> full logits
               P = softmax(causal(full))                   (both cores, dup)
               out_half = P @ v                            [T, H/2]
    host: concat out halves along H.

This is the zero-duplication split for the matmul FLOPs (10.74 GMAC/core vs
15.57 for query-split with duplicated k/v projections); only the cheap
exp/transpose work is duplicated. The logit exchange rides the dedicated
collective hardware (TOPSP/SDMA) and overlaps with the v projection.

The q/k projections and the QK logit matmul run in fp8e4m3 with
perf_mode=DoubleRow (2 fp8 weights per PE cell -> 2x matmul throughput);
weights are host-prescaled by 256 to clear the e4m3 subnormal range and the
1/256 is folded into the PSUM-drain scale. The v projection and PV matmul
stay bf16: fp8 error there lands directly in the output (measured ~3x over
the correctness budget), while q/k/logit quantization only perturbs softmax
logits by ~0.01. f32 PSUM accumulation everywhere.
"""

import math
import os

import numpy as np
import ml_dtypes

P = 128
B, T, C, H = 4, 2048, 1024, 2048
HH = H // 2   # head cols per core
HS = 256      # head columns per weight strip
# fp8 config (defaults are the shipped configuration; env is dev-only)
PROJ8 = os.environ.get("TP_PROJ8", "0") == "1"    # q/k projections fp8-DR
QK8 = os.environ.get("TP_QK8", "1") == "1"        # QK logit matmul fp8-DR
STAGE8 = os.environ.get("TP_STAGE8", "1") == "1"  # collective staging dtype
WSCALE = 256.0  # host pre-scale on Wq/Wk: lifts |W|<=2**-5 out of e4m3
                # subnormals; exact 2**-8 unscale folded into the PSUM drain
# Masked logits must survive a bf16 -> fp8e4m3 staging cast without hitting
# inf (e4m3 max 240). Both pair cores add the mask, so masked entries sum to
# -400 and exp underflows to exactly 0.
NEG = -200.0


def make_masks(t: int) -> np.ndarray:
    """masks[slot] : [P, P] additive mask for the diagonal key block of
    each query slot (other blocks are fully visible)."""
    nb = t // P
    masks = np.zeros((nb, P, P), dtype=np.float32)
    col = np.arange(P)[None, :]
    row = np.arange(P)[:, None]
    masks[:] = np.where(col <= row, 0.0, NEG)[None]
    return masks


def build_nc(t: int = T, c: int = C, h: int = H, reps: int = 1, phases: str = "ABC"):
    import concourse.bass as bass
    import concourse.mybir as mybir
    import concourse.tile as tile
    from concourse import bacc
    from concourse.masks import make_identity
    from concourse.tile_rust import add_dep_helper

    f32 = mybir.dt.float32
    bf16 = mybir.dt.bfloat16
    f8 = mybir.dt.float8e4
    DR = mybir.MatmulPerfMode.DoubleRow
    stage_dt = f8 if STAGE8 else bf16
    qk_dt = f8 if QK8 else bf16
    w_dt = f8 if PROJ8 else bf16

    hh2 = h // 2          # head cols per core
    nb = t // P           # key/query blocks
    ck = c // P           # contraction chunks
    hk = hh2 // P         # head chunks per core
    ts = 512              # t columns per projection matmul
    nhs = hh2 // HS       # weight strips
    scale = float(c) ** -0.5
    ncausal = nb * (nb + 1) // 2          # causal key blocks, all slots
    stg_elems = ncausal * P * P

    nc = bacc.Bacc("TRN2", target_bir_lowering=False, debug=False, num_devices=8)

    # x / mask / biases arrive pre-shuffled to the SBUF layout so every load
    # is one contiguous descriptor per partition: a "(k p) t -> p k t" gather
    # here costs ~19ns/descriptor of DGE time (4.9us per x chunk) and stalled
    # the PE ~17us at startup.
    xT = nc.dram_tensor("xT", [P, ck, t], bf16, kind="ExternalInput").ap()
    if PROJ8:
        xT8 = nc.dram_tensor("xT8", [P, ck, t], f8, kind="ExternalInput").ap()
    wq = nc.dram_tensor("wq", [nhs, P, ck, HS], w_dt, kind="ExternalInput").ap()
    wk = nc.dram_tensor("wk", [nhs, P, ck, HS], w_dt, kind="ExternalInput").ap()
    wv = nc.dram_tensor("wv", [nhs, P, ck, HS], bf16, kind="ExternalInput").ap()
    bq = nc.dram_tensor("bq", [P, hk], f32, kind="ExternalInput").ap()
    bk = nc.dram_tensor("bk", [P, hk], f32, kind="ExternalInput").ap()
    bv = nc.dram_tensor("bv", [hh2], f32, kind="ExternalInput").ap()
    mask = nc.dram_tensor("mask", [P, nb, P], bf16, kind="ExternalInput").ap()
    out = nc.dram_tensor("out", [t, hh2], f32, kind="ExternalOutput").ap()

    # Split the logit exchange into chunks so softmax/PV of early slots can
    # start while later slots are still in flight. Chunk g covers slots
    # [CHUNKS[g], CHUNKS[g+1]).
    _ck = os.environ.get("TP_CHUNKS", "8")
    CHUNKS = sorted({0, nb} | {int(v) for v in _ck.split(",") if v})
    CHUNKS = [v for v in CHUNKS if 0 <= v <= nb]

    def chunk_of(pb):
        for g in range(len(CHUNKS) - 1):
            if CHUNKS[g] <= pb < CHUNKS[g + 1]:
                return g
        raise AssertionError

    def chunk_span(g):
        """(elem offset, elem count) of chunk g in the packed staging buffer."""
        lo, hi = CHUNKS[g], CHUNKS[g + 1]
        off = lo * (lo + 1) // 2 * P * P
        end = hi * (hi + 1) // 2 * P * P
        return off, end - off

    def stg_slot(stg, pb):
        """[P, s_end] view of packed causal slot pb in a flat staging tile."""
        off = pb * (pb + 1) // 2 * P * P
        s_end = (pb + 1) * P
        return stg[off:off + P * s_end].rearrange("(p s) -> p s", s=s_end)

    with tile.TileContext(nc) as tc:
        with (
            tc.tile_pool(name="singles", bufs=1) as singles,
            tc.tile_pool(name="resident", bufs=1) as resident,
            tc.tile_pool(name="wp", bufs=2) as wp,
            tc.tile_pool(name="stgp", bufs=2) as stgp,
            tc.tile_pool(name="mrgp", bufs=4) as mrgp,
            tc.tile_pool(name="ptp", bufs=4) as ptp,
            tc.tile_pool(name="pttp", bufs=2) as pttp,
            tc.tile_pool(name="otp", bufs=2) as otp,
            tc.tile_pool(name="stats", bufs=8) as stats,
            tc.tile_pool(name="dram", bufs=1, space="DRAM") as dram,
            tc.tile_pool(name="ab_ps", bufs=4, space="PSUM") as ab_ps,
            tc.tile_pool(name="pt_ps", bufs=2, space="PSUM") as pt_ps,
        ):
            # Keep the startup DMAs off the sync queue (it carries the weight
            # strips, and each trigger costs ~1us of queue time): biases and
            # mask ride the scalar queue, bulk x loads ride vector/gpsimd.
            ident16 = singles.tile([P, P], bf16)
            make_identity(nc, ident16)
            bq_t = singles.tile([P, hk], f32)
            nc.scalar.dma_start(out=bq_t, in_=bq)
            bk_t = singles.tile([P, hk], f32)
            nc.scalar.dma_start(out=bk_t, in_=bk)
            # bv/mask are consumed late (v drains / QK staging); their DMAs
            # are emitted inside rep 0's phase A so they don't delay the x
            # chunk loads sharing the scalar queue.
            bv_t = singles.tile([P, hh2], f32)
            mask_t = singles.tile([P, nb, P], bf16)

            xt = resident.tile([P, ck, t], bf16)
            if PROJ8:
                xt8 = resident.tile([P, ck, t], f8)
            kt = resident.tile([P, hk, t], qk_dt)
            qt = resident.tile([P, hk, t], qk_dt)
            vt = resident.tile([P, nb, hh2], bf16)

            ngrp = len(CHUNKS) - 1
            # per buffer-parity history for cross-rep WAR/RAW fencing
            hist_ar = [[None] * ngrp, [None] * ngrp]
            mrg_of: dict = {}   # rep -> list of merge-DMA insts (C consumers)
            nstr = hh2 // 512

            def emit_C(crep, stg_out_c, ar_insts_c):
                """Softmax + PV + out for rep `crep` (may lag the main loop
                by one rep: software pipelining hides the AllReduce).

                Softmax prep (merge + exp + reciprocal) is emitted PREP_AHEAD
                slots ahead of the PV pass: DVE and ACT are in-order engines,
                so interleaving prep with the PV drains would chain slot
                pb's exp behind slot pb-1's PV completion and stall the PE
                by ~240ns x scn every slot."""
                mrg_list = mrg_of.setdefault(crep, [])
                preps = {}

                def emit_prep(pb):
                    scn = pb + 1
                    s_end = scn * P
                    g = chunk_of(pb)
                    offg, cntg = chunk_span(g)
                    d_in = pb * (pb + 1) // 2 * P * P - offg
                    # AllGather output is rank-major [2, cnt] per chunk:
                    # read both pair halves and sum them locally.
                    mrg8 = mrgp.tile([P, 2, t], stage_dt,
                                     name=f"mrg8{crep}_{pb}", tag="mrg8")
                    # One batched merge read on the sync queue (each DMA
                    # trigger costs ~1us of queue time; gpsimd is avoided —
                    # queuing reads behind a pending collective trigger
                    # deadlocks the CC mesh for ~130us). Both pair halves
                    # come in one 3D access pattern: [partition, rank, s].
                    base = 2 * offg + d_in
                    src = bass.AP(
                        tensor=stg_out_c.tensor,
                        offset=stg_out_c.offset + base,
                        ap=[[s_end, P], [cntg, 2], [1, s_end]])
                    md = nc.sync.dma_start(out=mrg8[:, :, :s_end], in_=src)
                    add_dep_helper(md.ins, ar_insts_c[g],
                                   reason="merge waits AG output")
                    mrg_list.append(md.ins)
                    mrg = mrgp.tile([P, t], bf16, name=f"mrg{crep}_{pb}",
                                    tag="mrg")
                    # merge-add on gpsimd (SBUF-only op, so Pool can do it):
                    # keeps DVE free for the just-in-time ptt copies below
                    nc.gpsimd.tensor_add(out=mrg[:, :s_end],
                                         in0=mrg8[:, 0, :s_end],
                                         in1=mrg8[:, 1, :s_end])
                    pt_t = ptp.tile([P, t], bf16, name=f"pt{crep}_{pb}",
                                    tag="pt")
                    rsum = stats.tile([P, 1], f32, name=f"rs{crep}_{pb}",
                                      tag="rs")
                    nc.scalar.activation(
                        out=pt_t[:, :s_end], in_=mrg[:, :s_end],
                        func=mybir.ActivationFunctionType.Exp,
                        accum_out=rsum)
                    # NOTE: the reciprocal is NOT taken here — a DVE op in
                    # the prep lets the scheduler interleave AG-gated work
                    # ahead of the v-projection PSUM drains on DVE, stalling
                    # the PE ~11us. It rides the PV pass instead.
                    preps[pb] = (pt_t, rsum)

                PREP_AHEAD = 4
                # chunks in order, but the FINAL chunk's slots run largest-
                # first: the kernel then ends on a small slot, trimming the
                # drain+out tail after the last matmul. (Each slot only
                # needs its own chunk's AllGather, so reordering within a
                # chunk is free.)
                order = []
                for g in range(len(CHUNKS) - 1):
                    span_slots = list(range(CHUNKS[g], CHUNKS[g + 1]))
                    order.extend(reversed(span_slots)
                                 if g == len(CHUNKS) - 2 else span_slots)
                for pb in order[:min(PREP_AHEAD, nb)]:
                    emit_prep(pb)
                for i, pb in enumerate(order):
                    scn = pb + 1
                    s_end = scn * P
                    pt_t, rsum = preps.pop(pb)
                    if i + PREP_AHEAD < nb:
                        emit_prep(order[i + PREP_AHEAD])
                    rinv = stats.tile([P, 1], f32, name=f"ri{crep}_{pb}",
                                      tag="ri")
                    nc.vector.reciprocal(rinv, rsum)
                    ptt = pttp.tile([P, nb, P], bf16, name=f"ptt{crep}_{pb}",
                                    tag="ptt")

                    def emit_tr(sc):
                        pps = pt_ps.tile([P, P], bf16, tag="pt_ps",
                                         name=f"pps{crep}_{pb}_{sc}")
                        nc.tensor.transpose(
                            pps, pt_t[:, sc * P:(sc + 1) * P], ident16)
                        # PSUM drain on DVE: ACT is busy with the lookahead
                        # slot's 1.9us exp, and the PE needs these
                        # just-in-time for the interleaved PV matmuls
                        nc.vector.tensor_scalar_mul(
                            out=ptt[:, sc, :], in0=pps, scalar1=1.0)

                    # PV accumulators come from the (phase-A/B-idle) ab_ps
                    # pool: its 4-buffer tag rotation double-buffers
                    # consecutive slots for free. Transposes interleave
                    # between PV matmuls so the ACT ptt copies hide behind
                    # the 2x-longer PV matmuls instead of pacing the PE.
                    pv = [ab_ps.tile([P, 512], f32, tag="ps",
                                     name=f"pv{n}_{crep}_{pb}")
                          for n in range(nstr)]
                    emit_tr(0)
                    if scn > 1:
                        emit_tr(1)
                    for sc in range(scn):
                        for n in range(nstr):
                            nc.tensor.matmul(
                                pv[n], lhsT=ptt[:, sc, :],
                                rhs=vt[:, sc, n * 512:(n + 1) * 512],
                                start=(sc == 0), stop=(sc == scn - 1))
                        if sc + 2 < scn:
                            emit_tr(sc + 2)
                    ot = otp.tile([P, hh2], f32, name=f"ot{crep}_{pb}",
                                  tag="ot")
                    # both PSUM drain halves on ACT: a drain on DVE sits
                    # ahead of the next slot's first ptt copy in DVE's
                    # in-order queue and costs ~650ns per slot; ACT's exp
                    # runs 4 slots ahead so it has the slack
                    nc.scalar.activation(
                        out=ot[:, :512], in_=pv[0],
                        func=mybir.ActivationFunctionType.Copy, scale=rinv)
                    nc.scalar.activation(
                        out=ot[:, 512:], in_=pv[1],
                        func=mybir.ActivationFunctionType.Copy, scale=rinv)
                    nc.gpsimd.dma_start(out=out[pb * P:(pb + 1) * P, :],
                                        in_=ot)

            pend = None  # (rep, stg_out, ar_insts) awaiting its C phase
            for rep in range(reps):
                par = rep % 2
                stg_in = dram.tile([stg_elems], stage_dt, name=f"stg_in{rep}",
                                   tag="stg_in", bufs=2)
                stg_out = dram.tile([2 * stg_elems], stage_dt,
                                    name=f"stg_out{rep}", tag="stg_out", bufs=2)
                stage_dmas = [[] for _ in range(ngrp)]
                ar_insts = [None] * ngrp

                if "A" in phases:
                    # first weight strip rides the sync queue ahead of the x
                    # chunks so the PE's first matmul gates only on ~3us of
                    # transfers
                    strip0 = wp.tile([P, ck, HS], w_dt,
                                     name=f"wkt_0_{rep}", tag="w")
                    nc.sync.dma_start(out=strip0, in_=wk[0])
                    # single-chunk x loads alternating sync/scalar queues:
                    # the first projection matmul needs only chunk 0, and the
                    # PE consumes one chunk per ~850ns while two queues
                    # deliver one per ~1.5us each.
                    proj_x, proj_src = (xt8, xT8) if PROJ8 else (xt, xT)
                    for g in range(ck):
                        # even chunks on scalar: its preamble is shorter than
                        # sync's (which also carries the strip-0 transfer),
                        # so chunk 0 lands first
                        eng = nc.scalar if g % 2 == 0 else nc.sync
                        eng.dma_start(
                            out=proj_x[:, g:g + 1, :],
                            in_=proj_src[:, g:g + 1, :])
                    if PROJ8:
                        # x in bf16 is only consumed by the v projection,
                        # well after the QK phase: one bulk transfer.
                        nc.gpsimd.dma_start(out=xt, in_=xT)
                    if rep == 0:
                        bv_bcast = bass.AP(tensor=bv.tensor, offset=bv.offset,
                                           ap=[[0, P], [1, hh2]])
                        nc.scalar.dma_start(out=bv_t, in_=bv_bcast)
                        nc.scalar.dma_start(out=mask_t, in_=mask)
                    # ---- k and q projections (weight-stationary) ----
                    drain_scale = 1.0 / WSCALE if PROJ8 else 1.0
                    for store, wten, bias in ((kt, wk, bk_t), (qt, wq, bq_t)):
                        for hsi in range(nhs):
                            if store is kt and hsi == 0:
                                w_t = strip0
                            else:
                                w_t = wp.tile(
                                    [P, ck, HS], w_dt,
                                    name=f"w{store.tensor.name}_{hsi}_{rep}",
                                    tag="w")
                                nc.sync.dma_start(out=w_t, in_=wten[hsi])
                            for h2 in range(HS // P):
                                hh = hsi * (HS // P) + h2
                                pss = [ab_ps.tile([P, ts], f32, tag="ps",
                                                  name=f"ps{rep}_{hsi}_{h2}_{tt}")
                                       for tt in range(t // ts)]
                                if PROJ8:
                                    # fp8 DoubleRow: two 128-row contraction
                                    # subtiles per pass, 2x PE throughput
                                    for g in range(ck // 2):
                                        for tt in range(t // ts):
                                            nc.tensor.matmul(
                                                pss[tt],
                                                lhsT=w_t[:, 2 * g:2 * g + 2,
                                                         h2 * P:(h2 + 1) * P],
                                                rhs=xt8[:, 2 * g:2 * g + 2,
                                                        tt * ts:(tt + 1) * ts],
                                                start=(g == 0),
                                                stop=(g == ck // 2 - 1),
                                                perf_mode=DR)
                                else:
                                    for cc in range(ck):
                                        for tt in range(t // ts):
                                            nc.tensor.matmul(
                                                pss[tt],
                                                lhsT=w_t[:, cc, h2 * P:(h2 + 1) * P],
                                                rhs=xt[:, cc, tt * ts:(tt + 1) * ts],
                                                start=(cc == 0), stop=(cc == ck - 1))
                                # Split the PSUM drains: k on DVE, q stays on
                                # ACT (only ACT/DVE may read PSUM). The 1/256
                                # weight unscale rides the drain for free.
                                for tt in range(t // ts):
                                    if store is kt:
                                        nc.vector.tensor_scalar(
                                            out=store[:, hh,
                                                      tt * ts:(tt + 1) * ts],
                                            in0=pss[tt],
                                            scalar1=drain_scale,
                                            scalar2=bias[:, hh:hh + 1],
                                            op0=mybir.AluOpType.mult,
                                            op1=mybir.AluOpType.add)
                                    else:
                                        nc.scalar.activation(
                                            out=store[:, hh,
                                                      tt * ts:(tt + 1) * ts],
                                            in_=pss[tt],
                                            func=mybir.ActivationFunctionType
                                            .Identity,
                                            scale=drain_scale,
                                            bias=bias[:, hh:hh + 1])

                if "B" in phases:
                    # ---- QK partial logits + stage + AllReduce ----
                    for pb in range(nb):
                        scn = pb + 1
                        s_end = scn * P
                        # single-pass PSUM drain straight to the staging
                        # dtype (scale folded in); the diagonal causal mask
                        # rides a small DVE add afterwards. One rounding,
                        # and half the ACT work of a bf16+cast pipeline.
                        stg = stgp.tile([P, t], stage_dt,
                                        name=f"stg{rep}_{pb}", tag="stg")
                        for ss in range(math.ceil(s_end / ts)):
                            w = min(ts, s_end - ss * ts)
                            ps = ab_ps.tile([P, ts], f32, tag="ps",
                                            name=f"qk{rep}_{pb}_{ss}")
                            if QK8:
                                for hg in range(hk // 2):
                                    nc.tensor.matmul(
                                        ps[:, :w],
                                        lhsT=qt[:, 2 * hg:2 * hg + 2,
                                                pb * P:(pb + 1) * P],
                                        rhs=kt[:, 2 * hg:2 * hg + 2,
                                               ss * ts:ss * ts + w],
                                        start=(hg == 0),
                                        stop=(hg == hk // 2 - 1),
                                        perf_mode=DR)
                            else:
                                for hh in range(hk):
                                    nc.tensor.matmul(
                                        ps[:, :w],
                                        lhsT=qt[:, hh, pb * P:(pb + 1) * P],
                                        rhs=kt[:, hh, ss * ts:ss * ts + w],
                                        start=(hh == 0), stop=(hh == hk - 1))
                            nc.scalar.activation(
                                out=stg[:, ss * ts:ss * ts + w], in_=ps[:, :w],
                                func=mybir.ActivationFunctionType.Copy,
                                scale=scale)
                        nc.vector.tensor_add(
                            out=stg[:, s_end - P:s_end],
                            in0=stg[:, s_end - P:s_end], in1=mask_t[:, pb, :])
                        g = chunk_of(pb)
                        # staging DMAs alternate sync/scalar queues: ~1us of
                        # trigger time each, and keeping the sync queue
                        # shallow lets the C-phase merge reads (also on sync)
                        # start promptly. gpsimd is avoided - queuing behind
                        # its pending collective triggers wedges the CC mesh.
                        eng = nc.sync if pb % 2 == 0 else nc.scalar
                        d = eng.dma_start(out=stg_slot(stg_in, pb),
                                          in_=stg[:, :s_end])
                        stage_dmas[g].append(d.ins)
                        if hist_ar[par][g] is not None:
                            # stg_in buffer reused 2 reps back: wait that AR
                            add_dep_helper(d.ins, hist_ar[par][g],
                                           reason="stage WAR vs old AR read")
                        if pb + 1 in CHUNKS:
                            off, cnt = chunk_span(g)
                            ar = nc.gpsimd.collective_compute(
                                kind="AllGather",
                                op=mybir.AluOpType.bypass,
                                replica_groups=[[0, 1], [2, 3], [4, 5], [6, 7]],
                                ins=[stg_in[off:off + cnt]],
                                outs=[stg_out[2 * off:2 * off + 2 * cnt]],
                            )
                            ar_insts[g] = ar.ins
                            for dins in stage_dmas[g]:
                                add_dep_helper(ar.ins, dins,
                                               reason="AR waits chunk staging")
                            for m in mrg_of.get(rep - 2, []):
                                add_dep_helper(ar.ins, m,
                                               reason="AR WAR vs old merge read")

                if "C" in phases and pend is not None:
                    # software-pipelined: rep r-1's softmax/PV runs here,
                    # covering rep r's AllReduce latency
                    emit_C(*pend)

                if "A" in phases:
                    # ---- v projection (after QK so the collective overlaps) ----
                    for hsi in range(nhs):
                        wv_t = wp.tile([P, ck, HS], bf16, name=f"wv{hsi}_{rep}",
                                       tag="w")
                        # scalar queue: sync is ~16us deep in staging
                        # triggers by now and would starve the v projection
                        nc.scalar.dma_start(out=wv_t, in_=wv[hsi])
                        for sb in range(nb):
                            ps = ab_ps.tile([P, HS], f32, tag="ps",
                                            name=f"v{rep}_{hsi}_{sb}")
                            for cc in range(ck):
                                nc.tensor.matmul(
                                    ps,
                                    lhsT=xt[:, cc, sb * P:(sb + 1) * P],
                                    rhs=wv_t[:, cc, :],
                                    start=(cc == 0), stop=(cc == ck - 1))
                            nc.vector.tensor_add(
                                out=vt[:, sb, hsi * HS:(hsi + 1) * HS], in0=ps,
                                in1=bv_t[:, hsi * HS:(hsi + 1) * HS])

                pend = (rep, stg_out, ar_insts)
                hist_ar[par] = ar_insts

            if "C" in phases and pend is not None:
                emit_C(*pend)

    nc.compile()
    return nc


class Runner:
    """Compiles the per-core program once and runs it on 8 cores via PJRT."""

    def __init__(self, t: int = T, c: int = C, h: int = H, reps: int = 1,
                 phases: str = "ABC"):
        import jax
        import concourse.mybir as mybir
        from concourse import bass2jax
        from jax.experimental.shard_map import shard_map
        from jax.sharding import Mesh, NamedSharding, PartitionSpec

        bass2jax.install_neuronx_cc_hook()
        self.jax = jax
        nc = build_nc(t, c, h, reps=reps, phases=phases)
        self.nc = nc
        self.n_cores = 8

        partition_name = (nc.partition_id_tensor.name
                          if nc.partition_id_tensor else None)
        in_names, out_names, out_avals, zero_outs = [], [], [], []
        for alloc in nc.m.functions[0].allocations:
            if not isinstance(alloc, mybir.MemoryLocationSet):
                continue
            name = alloc.memorylocations[0].name
            if alloc.kind == "ExternalInput":
                if name != partition_name:
                    in_names.append(name)
            elif alloc.kind == "ExternalOutput":
                shape = tuple(alloc.tensor_shape)
                dtype = mybir.dt.np(alloc.dtype)
                out_names.append(name)
                out_avals.append(jax.core.ShapedArray(shape, dtype))
                zero_outs.append(np.zeros(shape, dtype))
        self.in_names = list(in_names)
        self.out_names = out_names
        self.out_avals = out_avals
        n_params = len(in_names)
        all_in_names = in_names + out_names
        if partition_name is not None:
            all_in_names = all_in_names + [partition_name]

        def _body(*args):
            operands = list(args)
            if partition_name is not None:
                operands.append(bass2jax.partition_id_tensor())
            outs = bass2jax._bass_exec_p.bind(
                *operands,
                out_avals=tuple(out_avals),
                in_names=tuple(all_in_names),
                out_names=tuple(out_names),
                lowering_input_output_aliases=(),
                sim_require_finite=True,
                sim_require_nnan=True,
                nc=nc,
            )
            return tuple(outs)

        devices = jax.devices()[:self.n_cores]
        self.mesh = Mesh(np.asarray(devices), ("core",))
        nspec = (PartitionSpec("core"),) * (n_params + len(out_names))
        self._fn = jax.jit(
            shard_map(_body, mesh=self.mesh, in_specs=nspec,
                      out_specs=(PartitionSpec("core"),) * len(out_names),
                      check_rep=False),
            keep_unused=True)
        self._sharding = NamedSharding(self.mesh, PartitionSpec("core"))
        self._zero_outs = zero_outs

    def stage(self, in_maps: list[dict[str, np.ndarray]]):
        jax = self.jax
        args = []
        for name in self.in_names:
            cat = np.concatenate([np.asarray(m[name]) for m in in_maps], axis=0)
            args.append(jax.device_put(cat, self._sharding))
        for z in self._zero_outs:
            cat = np.zeros((self.n_cores * z.shape[0], *z.shape[1:]), z.dtype)
            args.append(jax.device_put(cat, self._sharding))
        return args

    def run_staged(self, args):
        return self._fn(*args)

    def __call__(self, in_maps: list[dict[str, np.ndarray]]):
        out_arrs = self.run_staged(self.stage(in_maps))
        self.jax.block_until_ready(out_arrs)
        return [
            {name: np.asarray(out_arrs[i]).reshape(
                self.n_cores, *self.out_avals[i].shape)[cid]
             for i, name in enumerate(self.out_names)}
            for cid in range(self.n_cores)
        ]


_runner_cache: dict = {}


def get_runner(t: int = T, c: int = C, h: int = H, reps: int = 1) -> Runner:
    key = (t, c, h, reps)
    if key not in _runner_cache:
        _runner_cache[key] = Runner(t, c, h, reps)
    return _runner_cache[key]


def _shuffle_w(Whalf, c, scale=1.0, dtype=ml_dtypes.bfloat16):
    """[c, hh] -> [hh//HS, P, c//P, HS] so each weight-strip DMA reads
    one contiguous block with >=2KB-per-partition descriptor lines."""
    hh = Whalf.shape[1]
    W = (np.asarray(Whalf, np.float32) * scale).reshape(c // P, P, hh // HS, HS)
    return np.ascontiguousarray(W.transpose(2, 1, 0, 3)).astype(dtype)


def make_in_maps(x, Wq, bq, Wk, bk, Wv, bv):
    """Build the 8 per-core input dicts from full inputs."""
    x = np.asarray(x, dtype=np.float32)
    t, c = x.shape[1], x.shape[2]
    h = np.asarray(Wq).shape[1]
    hh = h // 2
    # [P, nb, P] so the device load is contiguous per partition
    masks = np.ascontiguousarray(
        make_masks(t).transpose(1, 0, 2)).astype(ml_dtypes.bfloat16)
    f8 = ml_dtypes.float8_e4m3  # TRN fp8e4 variant (max 240, has inf)
    wsc, wdt = (WSCALE, f8) if PROJ8 else (1.0, ml_dtypes.bfloat16)
    ck, hk = c // P, hh // P
    in_maps = []
    for core in range(8):
        b, half = divmod(core, 2)
        cols = slice(half * hh, (half + 1) * hh)
        # x.T pre-shuffled to the SBUF tile layout [P, ck, t]
        xTr = np.ascontiguousarray(x[b].T.reshape(ck, P, t).transpose(1, 0, 2))
        im = {
            "xT": xTr.astype(ml_dtypes.bfloat16),
            "wq": _shuffle_w(np.asarray(Wq, np.float32)[:, cols], c, wsc, wdt),
            "wk": _shuffle_w(np.asarray(Wk, np.float32)[:, cols], c, wsc, wdt),
            "wv": _shuffle_w(np.asarray(Wv, np.float32)[:, cols], c),
            "bq": np.ascontiguousarray(
                np.asarray(bq, np.float32)[cols].reshape(hk, P).T),
            "bk": np.ascontiguousarray(
                np.asarray(bk, np.float32)[cols].reshape(hk, P).T),
            "bv": np.asarray(bv, np.float32)[cols],
            "mask": masks,
        }
        if PROJ8:
            im["xT8"] = xTr.astype(f8)
        in_maps.append(im)
    return in_maps


def assemble(results, t, h):
    """Concat per-core [t, h/2] outputs back to [B, t, h]."""
    hh = h // 2
    out = np.empty((B, t, h), dtype=np.float32)
    for core in range(8):
        b, half = divmod(core, 2)
        out[b][:, half * hh:(half + 1) * hh] = results[core]["out"]
    return out


def kernel(x, Wq, bq, Wk, bk, Wv, bv):
    t, c, h = x.shape[1], x.shape[2], Wq.shape[1]
    runner = get_runner(t, c, h)
    results = runner(make_in_maps(x, Wq, bq, Wk, bk, Wv, bv))
    return assemble(results, t, h)



# revision 49
# speedup vs baseline: 1.0062x; 1.0062x over previous
"""Trainium2 Bass kernel for a single-head causal attention block.

Head-split (tensor-parallel) sharding over 8 NeuronCores: core = 2*b + half.
Each core handles one batch and HALF of the head dimension (1024 of 2048)
for ALL T queries/keys:

    per core:  q,k,v = x @ W*[:, half] + b*[half]          [T, H/2]
               partial[t, s] = (q @ k.T) * C**-0.5         [T, T] causal
               AllReduce(partial, pair) -> full logits
               P = softmax(causal(full))                   (both cores, dup)
               out_half = P @ v                            [T, H/2]
    host: concat out halves along H.

This is the zero-duplication split for the matmul FLOPs (10.74 GMAC/core vs
15.57 for query-split with duplicated k/v projections); only the cheap
exp/transpose work is duplicated. The logit exchange rides the dedicated
collective hardware (TOPSP/SDMA) and overlaps with the v projection.

The q/k projections and the QK logit matmul run in fp8e4m3 with
perf_mode=DoubleRow (2 fp8 weights per PE cell -> 2x matmul throughput);
weights are host-prescaled by 256 to clear the e4m3 subnormal range and the
1/256 is folded into the PSUM-drain scale. The v projection and PV matmul
stay bf16: fp8 error there lands directly in the output (measured ~3x over
the correctness budget), while q/k/logit quantization only perturbs softmax
logits by ~0.01. f32 PSUM accumulation everywhere.
"""

import math
import os

import numpy as np
import ml_dtypes

P = 128
B, T, C, H = 4, 2048, 1024, 2048
HH = H // 2   # head cols per core
HS = 256      # head columns per weight strip
# fp8 config (defaults are the shipped configuration; env is dev-only)
PROJ8 = os.environ.get("TP_PROJ8", "0") == "1"    # q/k projections fp8-DR
QK8 = os.environ.get("TP_QK8", "1") == "1"        # QK logit matmul fp8-DR
STAGE8 = os.environ.get("TP_STAGE8", "1") == "1"  # collective staging dtype
WSCALE = 256.0  # host pre-scale on Wq/Wk: lifts |W|<=2**-5 out of e4m3
                # subnormals; exact 2**-8 unscale folded into the PSUM drain
# Masked logits must survive a bf16 -> fp8e4m3 staging cast without hitting
# inf (e4m3 max 240). Both pair cores add the mask, so masked entries sum to
# -400 and exp underflows to exactly 0.
NEG = -200.0


def make_masks(t: int) -> np.ndarray:
    """masks[slot] : [P, P] additive mask for the diagonal key block of
    each query slot (other blocks are fully visible)."""
    nb = t // P
    masks = np.zeros((nb, P, P), dtype=np.float32)
    col = np.arange(P)[None, :]
    row = np.arange(P)[:, None]
    masks[:] = np.where(col <= row, 0.0, NEG)[None]
    return masks


def build_nc(t: int = T, c: int = C, h: int = H, reps: int = 1, phases: str = "ABC"):
    import concourse.bass as bass
    import concourse.mybir as mybir
    import concourse.tile as tile
    from concourse import bacc
    from concourse.masks import make_identity
    from concourse.tile_rust import add_dep_helper

    f32 = mybir.dt.float32
    bf16 = mybir.dt.bfloat16
    f8 = mybir.dt.float8e4
    DR = mybir.MatmulPerfMode.DoubleRow
    stage_dt = f8 if STAGE8 else bf16
    qk_dt = f8 if QK8 else bf16
    w_dt = f8 if PROJ8 else bf16

    hh2 = h // 2          # head cols per core
    nb = t // P           # key/query blocks
    ck = c // P           # contraction chunks
    hk = hh2 // P         # head chunks per core
    ts = 512              # t columns per projection matmul
    nhs = hh2 // HS       # weight strips
    scale = float(c) ** -0.5
    ncausal = nb * (nb + 1) // 2          # causal key blocks, all slots
    stg_elems = ncausal * P * P

    nc = bacc.Bacc("TRN2", target_bir_lowering=False, debug=False, num_devices=8)

    # x / mask / biases arrive pre-shuffled to the SBUF layout so every load
    # is one contiguous descriptor per partition: a "(k p) t -> p k t" gather
    # here costs ~19ns/descriptor of DGE time (4.9us per x chunk) and stalled
    # the PE ~17us at startup.
    xT = nc.dram_tensor("xT", [P, ck, t], bf16, kind="ExternalInput").ap()
    if PROJ8:
        xT8 = nc.dram_tensor("xT8", [P, ck, t], f8, kind="ExternalInput").ap()
    wq = nc.dram_tensor("wq", [nhs, P, ck, HS], w_dt, kind="ExternalInput").ap()
    wk = nc.dram_tensor("wk", [nhs, P, ck, HS], w_dt, kind="ExternalInput").ap()
    wv = nc.dram_tensor("wv", [nhs, P, ck, HS], bf16, kind="ExternalInput").ap()
    bq = nc.dram_tensor("bq", [P, hk], f32, kind="ExternalInput").ap()
    bk = nc.dram_tensor("bk", [P, hk], f32, kind="ExternalInput").ap()
    bv = nc.dram_tensor("bv", [hh2], f32, kind="ExternalInput").ap()
    mask = nc.dram_tensor("mask", [P, nb, P], bf16, kind="ExternalInput").ap()
    out = nc.dram_tensor("out", [t, hh2], f32, kind="ExternalOutput").ap()

    # Split the logit exchange into chunks so softmax/PV of early slots can
    # start while later slots are still in flight. Chunk g covers slots
    # [CHUNKS[g], CHUNKS[g+1]).
    _ck = os.environ.get("TP_CHUNKS", "8")
    CHUNKS = sorted({0, nb} | {int(v) for v in _ck.split(",") if v})
    CHUNKS = [v for v in CHUNKS if 0 <= v <= nb]

    def chunk_of(pb):
        for g in range(len(CHUNKS) - 1):
            if CHUNKS[g] <= pb < CHUNKS[g + 1]:
                return g
        raise AssertionError

    def chunk_span(g):
        """(elem offset, elem count) of chunk g in the packed staging buffer."""
        lo, hi = CHUNKS[g], CHUNKS[g + 1]
        off = lo * (lo + 1) // 2 * P * P
        end = hi * (hi + 1) // 2 * P * P
        return off, end - off

    def stg_slot(stg, pb):
        """[P, s_end] view of packed causal slot pb in a flat staging tile."""
        off = pb * (pb + 1) // 2 * P * P
        s_end = (pb + 1) * P
        return stg[off:off + P * s_end].rearrange("(p s) -> p s", s=s_end)

    with tile.TileContext(nc) as tc:
        with (
            tc.tile_pool(name="singles", bufs=1) as singles,
            tc.tile_pool(name="resident", bufs=1) as resident,
            tc.tile_pool(name="wp", bufs=2) as wp,
            tc.tile_pool(name="stgp", bufs=2) as stgp,
            tc.tile_pool(name="mrgp", bufs=4) as mrgp,
            tc.tile_pool(name="ptp", bufs=4) as ptp,
            tc.tile_pool(name="pttp", bufs=2) as pttp,
            tc.tile_pool(name="otp", bufs=2) as otp,
            tc.tile_pool(name="stats", bufs=8) as stats,
            tc.tile_pool(name="dram", bufs=1, space="DRAM") as dram,
            tc.tile_pool(name="ab_ps", bufs=4, space="PSUM") as ab_ps,
            tc.tile_pool(name="pt_ps", bufs=2, space="PSUM") as pt_ps,
        ):
            # Keep the startup DMAs off the sync queue (it carries the weight
            # strips, and each trigger costs ~1us of queue time): biases and
            # mask ride the scalar queue, bulk x loads ride vector/gpsimd.
            ident16 = singles.tile([P, P], bf16)
            make_identity(nc, ident16)
            bq_t = singles.tile([P, hk], f32)
            nc.scalar.dma_start(out=bq_t, in_=bq)
            bk_t = singles.tile([P, hk], f32)
            nc.scalar.dma_start(out=bk_t, in_=bk)
            # bv/mask are consumed late (v drains / QK staging); their DMAs
            # are emitted inside rep 0's phase A so they don't delay the x
            # chunk loads sharing the scalar queue.
            bv_t = singles.tile([P, hh2], f32)
            mask_t = singles.tile([P, nb, P], bf16)

            xt = resident.tile([P, ck, t], bf16)
            if PROJ8:
                xt8 = resident.tile([P, ck, t], f8)
            kt = resident.tile([P, hk, t], qk_dt)
            qt = resident.tile([P, hk, t], qk_dt)
            vt = resident.tile([P, nb, hh2], bf16)

            ngrp = len(CHUNKS) - 1
            # per buffer-parity history for cross-rep WAR/RAW fencing
            hist_ar = [[None] * ngrp, [None] * ngrp]
            mrg_of: dict = {}   # rep -> list of merge-DMA insts (C consumers)
            nstr = hh2 // 512

            def emit_C(crep, stg_out_c, ar_insts_c):
                """Softmax + PV + out for rep `crep` (may lag the main loop
                by one rep: software pipelining hides the AllReduce).

                Softmax prep (merge + exp + reciprocal) is emitted PREP_AHEAD
                slots ahead of the PV pass: DVE and ACT are in-order engines,
                so interleaving prep with the PV drains would chain slot
                pb's exp behind slot pb-1's PV completion and stall the PE
                by ~240ns x scn every slot."""
                mrg_list = mrg_of.setdefault(crep, [])
                preps = {}

                def emit_prep(pb):
                    scn = pb + 1
                    s_end = scn * P
                    g = chunk_of(pb)
                    offg, cntg = chunk_span(g)
                    d_in = pb * (pb + 1) // 2 * P * P - offg
                    # AllGather output is rank-major [2, cnt] per chunk:
                    # read both pair halves and sum them locally.
                    mrg8 = mrgp.tile([P, 2, t], stage_dt,
                                     name=f"mrg8{crep}_{pb}", tag="mrg8")
                    # One batched merge read on the sync queue (each DMA
                    # trigger costs ~1us of queue time; gpsimd is avoided —
                    # queuing reads behind a pending collective trigger
                    # deadlocks the CC mesh for ~130us). Both pair halves
                    # come in one 3D access pattern: [partition, rank, s].
                    base = 2 * offg + d_in
                    src = bass.AP(
                        tensor=stg_out_c.tensor,
                        offset=stg_out_c.offset + base,
                        ap=[[s_end, P], [cntg, 2], [1, s_end]])
                    md = nc.sync.dma_start(out=mrg8[:, :, :s_end], in_=src)
                    add_dep_helper(md.ins, ar_insts_c[g],
                                   reason="merge waits AG output")
                    mrg_list.append(md.ins)
                    mrg = mrgp.tile([P, t], bf16, name=f"mrg{crep}_{pb}",
                                    tag="mrg")
                    # merge-add on gpsimd (SBUF-only op, so Pool can do it):
                    # keeps DVE free for the just-in-time ptt copies below
                    nc.gpsimd.tensor_add(out=mrg[:, :s_end],
                                         in0=mrg8[:, 0, :s_end],
                                         in1=mrg8[:, 1, :s_end])
                    pt_t = ptp.tile([P, t], bf16, name=f"pt{crep}_{pb}",
                                    tag="pt")
                    rsum = stats.tile([P, 1], f32, name=f"rs{crep}_{pb}",
                                      tag="rs")
                    nc.scalar.activation(
                        out=pt_t[:, :s_end], in_=mrg[:, :s_end],
                        func=mybir.ActivationFunctionType.Exp,
                        accum_out=rsum)
                    # NOTE: the reciprocal is NOT taken here — a DVE op in
                    # the prep lets the scheduler interleave AG-gated work
                    # ahead of the v-projection PSUM drains on DVE, stalling
                    # the PE ~11us. It rides the PV pass instead.
                    preps[pb] = (pt_t, rsum)

                PREP_AHEAD = 4
                order = list(range(nb))
                for pb in order[:min(PREP_AHEAD, nb)]:
                    emit_prep(pb)
                for i, pb in enumerate(order):
                    scn = pb + 1
                    s_end = scn * P
                    pt_t, rsum = preps.pop(pb)
                    if i + PREP_AHEAD < nb:
                        emit_prep(order[i + PREP_AHEAD])
                    rinv = stats.tile([P, 1], f32, name=f"ri{crep}_{pb}",
                                      tag="ri")
                    nc.vector.reciprocal(rinv, rsum)
                    ptt = pttp.tile([P, nb, P], bf16, name=f"ptt{crep}_{pb}",
                                    tag="ptt")

                    def emit_tr(sc):
                        pps = pt_ps.tile([P, P], bf16, tag="pt_ps",
                                         name=f"pps{crep}_{pb}_{sc}")
                        nc.tensor.transpose(
                            pps, pt_t[:, sc * P:(sc + 1) * P], ident16)
                        # PSUM drain on DVE: ACT is busy with the lookahead
                        # slot's 1.9us exp, and the PE needs these
                        # just-in-time for the interleaved PV matmuls
                        nc.vector.tensor_scalar_mul(
                            out=ptt[:, sc, :], in0=pps, scalar1=1.0)

                    # PV accumulators come from the (phase-A/B-idle) ab_ps
                    # pool: its 4-buffer tag rotation double-buffers
                    # consecutive slots for free. Transposes interleave
                    # between PV matmuls so the ACT ptt copies hide behind
                    # the 2x-longer PV matmuls instead of pacing the PE.
                    pv = [ab_ps.tile([P, 512], f32, tag="ps",
                                     name=f"pv{n}_{crep}_{pb}")
                          for n in range(nstr)]
                    emit_tr(0)
                    if scn > 1:
                        emit_tr(1)
                    for sc in range(scn):
                        for n in range(nstr):
                            nc.tensor.matmul(
                                pv[n], lhsT=ptt[:, sc, :],
                                rhs=vt[:, sc, n * 512:(n + 1) * 512],
                                start=(sc == 0), stop=(sc == scn - 1))
                        if sc + 2 < scn:
                            emit_tr(sc + 2)
                    ot = otp.tile([P, hh2], f32, name=f"ot{crep}_{pb}",
                                  tag="ot")
                    # split the PSUM drain across DVE and ACT: the pv banks
                    # free in half the time, so the next slot's matmuls
                    # start sooner
                    nc.vector.tensor_scalar_mul(
                        out=ot[:, :512], in0=pv[0], scalar1=rinv)
                    nc.scalar.activation(
                        out=ot[:, 512:], in_=pv[1],
                        func=mybir.ActivationFunctionType.Copy, scale=rinv)
                    nc.gpsimd.dma_start(out=out[pb * P:(pb + 1) * P, :],
                                        in_=ot)

            pend = None  # (rep, stg_out, ar_insts) awaiting its C phase
            for rep in range(reps):
                par = rep % 2
                stg_in = dram.tile([stg_elems], stage_dt, name=f"stg_in{rep}",
                                   tag="stg_in", bufs=2)
                stg_out = dram.tile([2 * stg_elems], stage_dt,
                                    name=f"stg_out{rep}", tag="stg_out", bufs=2)
                stage_dmas = [[] for _ in range(ngrp)]
                ar_insts = [None] * ngrp

                if "A" in phases:
                    # first weight strip rides the sync queue ahead of the x
                    # chunks so the PE's first matmul gates only on ~3us of
                    # transfers
                    strip0 = wp.tile([P, ck, HS], w_dt,
                                     name=f"wkt_0_{rep}", tag="w")
                    nc.sync.dma_start(out=strip0, in_=wk[0])
                    # single-chunk x loads alternating sync/scalar queues:
                    # the first projection matmul needs only chunk 0, and the
                    # PE consumes one chunk per ~850ns while two queues
                    # deliver one per ~1.5us each.
                    proj_x, proj_src = (xt8, xT8) if PROJ8 else (xt, xT)
                    for g in range(ck):
                        eng = nc.sync if g % 2 == 0 else nc.scalar
                        eng.dma_start(
                            out=proj_x[:, g:g + 1, :],
                            in_=proj_src[:, g:g + 1, :])
                    if PROJ8:
                        # x in bf16 is only consumed by the v projection,
                        # well after the QK phase: one bulk transfer.
                        nc.gpsimd.dma_start(out=xt, in_=xT)
                    if rep == 0:
                        bv_bcast = bass.AP(tensor=bv.tensor, offset=bv.offset,
                                           ap=[[0, P], [1, hh2]])
                        nc.scalar.dma_start(out=bv_t, in_=bv_bcast)
                        nc.scalar.dma_start(out=mask_t, in_=mask)
                    # ---- k and q projections (weight-stationary) ----
                    drain_scale = 1.0 / WSCALE if PROJ8 else 1.0
                    for store, wten, bias in ((kt, wk, bk_t), (qt, wq, bq_t)):
                        for hsi in range(nhs):
                            if store is kt and hsi == 0:
                                w_t = strip0
                            else:
                                w_t = wp.tile(
                                    [P, ck, HS], w_dt,
                                    name=f"w{store.tensor.name}_{hsi}_{rep}",
                                    tag="w")
                                nc.sync.dma_start(out=w_t, in_=wten[hsi])
                            for h2 in range(HS // P):
                                hh = hsi * (HS // P) + h2
                                pss = [ab_ps.tile([P, ts], f32, tag="ps",
                                                  name=f"ps{rep}_{hsi}_{h2}_{tt}")
                                       for tt in range(t // ts)]
                                if PROJ8:
                                    # fp8 DoubleRow: two 128-row contraction
                                    # subtiles per pass, 2x PE throughput
                                    for g in range(ck // 2):
                                        for tt in range(t // ts):
                                            nc.tensor.matmul(
                                                pss[tt],
                                                lhsT=w_t[:, 2 * g:2 * g + 2,
                                                         h2 * P:(h2 + 1) * P],
                                                rhs=xt8[:, 2 * g:2 * g + 2,
                                                        tt * ts:(tt + 1) * ts],
                                                start=(g == 0),
                                                stop=(g == ck // 2 - 1),
                                                perf_mode=DR)
                                else:
                                    for cc in range(ck):
                                        for tt in range(t // ts):
                                            nc.tensor.matmul(
                                                pss[tt],
                                                lhsT=w_t[:, cc, h2 * P:(h2 + 1) * P],
                                                rhs=xt[:, cc, tt * ts:(tt + 1) * ts],
                                                start=(cc == 0), stop=(cc == ck - 1))
                                # Split the PSUM drains: k on DVE, q stays on
                                # ACT (only ACT/DVE may read PSUM). The 1/256
                                # weight unscale rides the drain for free.
                                for tt in range(t // ts):
                                    if store is kt:
                                        nc.vector.tensor_scalar(
                                            out=store[:, hh,
                                                      tt * ts:(tt + 1) * ts],
                                            in0=pss[tt],
                                            scalar1=drain_scale,
                                            scalar2=bias[:, hh:hh + 1],
                                            op0=mybir.AluOpType.mult,
                                            op1=mybir.AluOpType.add)
                                    else:
                                        nc.scalar.activation(
                                            out=store[:, hh,
                                                      tt * ts:(tt + 1) * ts],
                                            in_=pss[tt],
                                            func=mybir.ActivationFunctionType
                                            .Identity,
                                            scale=drain_scale,
                                            bias=bias[:, hh:hh + 1])

                if "B" in phases:
                    # ---- QK partial logits + stage + AllReduce ----
                    for pb in range(nb):
                        scn = pb + 1
                        s_end = scn * P
                        # single-pass PSUM drain straight to the staging
                        # dtype (scale folded in); the diagonal causal mask
                        # rides a small DVE add afterwards. One rounding,
                        # and half the ACT work of a bf16+cast pipeline.
                        stg = stgp.tile([P, t], stage_dt,
                                        name=f"stg{rep}_{pb}", tag="stg")
                        for ss in range(math.ceil(s_end / ts)):
                            w = min(ts, s_end - ss * ts)
                            ps = ab_ps.tile([P, ts], f32, tag="ps",
                                            name=f"qk{rep}_{pb}_{ss}")
                            if QK8:
                                for hg in range(hk // 2):
                                    nc.tensor.matmul(
                                        ps[:, :w],
                                        lhsT=qt[:, 2 * hg:2 * hg + 2,
                                                pb * P:(pb + 1) * P],
                                        rhs=kt[:, 2 * hg:2 * hg + 2,
                                               ss * ts:ss * ts + w],
                                        start=(hg == 0),
                                        stop=(hg == hk // 2 - 1),
                                        perf_mode=DR)
                            else:
                                for hh in range(hk):
                                    nc.tensor.matmul(
                                        ps[:, :w],
                                        lhsT=qt[:, hh, pb * P:(pb + 1) * P],
                                        rhs=kt[:, hh, ss * ts:ss * ts + w],
                                        start=(hh == 0), stop=(hh == hk - 1))
                            nc.scalar.activation(
                                out=stg[:, ss * ts:ss * ts + w], in_=ps[:, :w],
                                func=mybir.ActivationFunctionType.Copy,
                                scale=scale)
                        nc.vector.tensor_add(
                            out=stg[:, s_end - P:s_end],
                            in0=stg[:, s_end - P:s_end], in1=mask_t[:, pb, :])
                        g = chunk_of(pb)
                        # staging DMAs alternate sync/scalar queues: ~1us of
                        # trigger time each, and keeping the sync queue
                        # shallow lets the C-phase merge reads (also on sync)
                        # start promptly. gpsimd is avoided - queuing behind
                        # its pending collective triggers wedges the CC mesh.
                        eng = nc.sync if pb % 2 == 0 else nc.scalar
                        d = eng.dma_start(out=stg_slot(stg_in, pb),
                                          in_=stg[:, :s_end])
                        stage_dmas[g].append(d.ins)
                        if hist_ar[par][g] is not None:
                            # stg_in buffer reused 2 reps back: wait that AR
                            add_dep_helper(d.ins, hist_ar[par][g],
                                           reason="stage WAR vs old AR read")
                        if pb + 1 in CHUNKS:
                            off, cnt = chunk_span(g)
                            ar = nc.gpsimd.collective_compute(
                                kind="AllGather",
                                op=mybir.AluOpType.bypass,
                                replica_groups=[[0, 1], [2, 3], [4, 5], [6, 7]],
                                ins=[stg_in[off:off + cnt]],
                                outs=[stg_out[2 * off:2 * off + 2 * cnt]],
                            )
                            ar_insts[g] = ar.ins
                            for dins in stage_dmas[g]:
                                add_dep_helper(ar.ins, dins,
                                               reason="AR waits chunk staging")
                            for m in mrg_of.get(rep - 2, []):
                                add_dep_helper(ar.ins, m,
                                               reason="AR WAR vs old merge read")

                if "C" in phases and pend is not None:
                    # software-pipelined: rep r-1's softmax/PV runs here,
                    # covering rep r's AllReduce latency
                    emit_C(*pend)

                if "A" in phases:
                    # ---- v projection (after QK so the collective overlaps) ----
                    for hsi in range(nhs):
                        wv_t = wp.tile([P, ck, HS], bf16, name=f"wv{hsi}_{rep}",
                                       tag="w")
                        # scalar queue: sync is ~16us deep in staging
                        # triggers by now and would starve the v projection
                        nc.scalar.dma_start(out=wv_t, in_=wv[hsi])
                        for sb in range(nb):
                            ps = ab_ps.tile([P, HS], f32, tag="ps",
                                            name=f"v{rep}_{hsi}_{sb}")
                            for cc in range(ck):
                                nc.tensor.matmul(
                                    ps,
                                    lhsT=xt[:, cc, sb * P:(sb + 1) * P],
                                    rhs=wv_t[:, cc, :],
                                    start=(cc == 0), stop=(cc == ck - 1))
                            nc.vector.tensor_add(
                                out=vt[:, sb, hsi * HS:(hsi + 1) * HS], in0=ps,
                                in1=bv_t[:, hsi * HS:(hsi + 1) * HS])

                pend = (rep, stg_out, ar_insts)
                hist_ar[par] = ar_insts

            if "C" in phases and pend is not None:
                emit_C(*pend)

    nc.compile()
    return nc


class Runner:
    """Compiles the per-core program once and runs it on 8 cores via PJRT."""

    def __init__(self, t: int = T, c: int = C, h: int = H, reps: int = 1,
                 phases: str = "ABC"):
        import jax
        import concourse.mybir as mybir
        from concourse import bass2jax
        from jax.experimental.shard_map import shard_map
        from jax.sharding import Mesh, NamedSharding, PartitionSpec

        bass2jax.install_neuronx_cc_hook()
        self.jax = jax
        nc = build_nc(t, c, h, reps=reps, phases=phases)
        self.nc = nc
        self.n_cores = 8

        partition_name = (nc.partition_id_tensor.name
                          if nc.partition_id_tensor else None)
        in_names, out_names, out_avals, zero_outs = [], [], [], []
        for alloc in nc.m.functions[0].allocations:
            if not isinstance(alloc, mybir.MemoryLocationSet):
                continue
            name = alloc.memorylocations[0].name
            if alloc.kind == "ExternalInput":
                if name != partition_name:
                    in_names.append(name)
            elif alloc.kind == "ExternalOutput":
                shape = tuple(alloc.tensor_shape)
                dtype = mybir.dt.np(alloc.dtype)
                out_names.append(name)
                out_avals.append(jax.core.ShapedArray(shape, dtype))
                zero_outs.append(np.zeros(shape, dtype))
        self.in_names = list(in_names)
        self.out_names = out_names
        self.out_avals = out_avals
        n_params = len(in_names)
        all_in_names = in_names + out_names
        if partition_name is not None:
            all_in_names = all_in_names + [partition_name]

        def _body(*args):
            operands = list(args)
            if partition_name is not None:
                operands.append(bass2jax.partition_id_tensor())
            outs = bass2jax._bass_exec_p.bind(
                *operands,
                out_avals=tuple(out_avals),
                in_names=tuple(all_in_names),
                out_names=tuple(out_names),
                lowering_input_output_aliases=(),
                sim_require_finite=True,
                sim_require_nnan=True,
                nc=nc,
            )
            return tuple(outs)

        devices = jax.devices()[:self.n_cores]
        self.mesh = Mesh(np.asarray(devices), ("core",))
        nspec = (PartitionSpec("core"),) * (n_params + len(out_names))
        self._fn = jax.jit(
            shard_map(_body, mesh=self.mesh, in_specs=nspec,
                      out_specs=(PartitionSpec("core"),) * len(out_names),
                      check_rep=False),
            keep_unused=True)
        self._sharding = NamedSharding(self.mesh, PartitionSpec("core"))
        self._zero_outs = zero_outs

    def stage(self, in_maps: list[dict[str, np.ndarray]]):
        jax = self.jax
        args = []
        for name in self.in_names:
            cat = np.concatenate([np.asarray(m[name]) for m in in_maps], axis=0)
            args.append(jax.device_put(cat, self._sharding))
        for z in self._zero_outs:
            cat = np.zeros((self.n_cores * z.shape[0], *z.shape[1:]), z.dtype)
            args.append(jax.device_put(cat, self._sharding))
        return args

    def run_staged(self, args):
        return self._fn(*args)

    def __call__(self, in_maps: list[dict[str, np.ndarray]]):
        out_arrs = self.run_staged(self.stage(in_maps))
        self.jax.block_until_ready(out_arrs)
        return [
            {name: np.asarray(out_arrs[i]).reshape(
                self.n_cores, *self.out_avals[i].shape)[cid]
             for i, name in enumerate(self.out_names)}
            for cid in range(self.n_cores)
        ]


_runner_cache: dict = {}


def get_runner(t: int = T, c: int = C, h: int = H, reps: int = 1) -> Runner:
    key = (t, c, h, reps)
    if key not in _runner_cache:
        _runner_cache[key] = Runner(t, c, h, reps)
    return _runner_cache[key]


def _shuffle_w(Whalf, c, scale=1.0, dtype=ml_dtypes.bfloat16):
    """[c, hh] -> [hh//HS, P, c//P, HS] so each weight-strip DMA reads
    one contiguous block with >=2KB-per-partition descriptor lines."""
    hh = Whalf.shape[1]
    W = (np.asarray(Whalf, np.float32) * scale).reshape(c // P, P, hh // HS, HS)
    return np.ascontiguousarray(W.transpose(2, 1, 0, 3)).astype(dtype)


def make_in_maps(x, Wq, bq, Wk, bk, Wv, bv):
    """Build the 8 per-core input dicts from full inputs."""
    x = np.asarray(x, dtype=np.float32)
    t, c = x.shape[1], x.shape[2]
    h = np.asarray(Wq).shape[1]
    hh = h // 2
    # [P, nb, P] so the device load is contiguous per partition
    masks = np.ascontiguousarray(
        make_masks(t).transpose(1, 0, 2)).astype(ml_dtypes.bfloat16)
    f8 = ml_dtypes.float8_e4m3  # TRN fp8e4 variant (max 240, has inf)
    wsc, wdt = (WSCALE, f8) if PROJ8 else (1.0, ml_dtypes.bfloat16)
    ck, hk = c // P, hh // P
    in_maps = []
    for core in range(8):
        b, half = divmod(core, 2)
        cols = slice(half * hh, (half + 1) * hh)
        # x.T pre-shuffled to the SBUF tile layout [P, ck, t]
        xTr = np.ascontiguousarray(x[b].T.reshape(ck, P, t).transpose(1, 0, 2))
        im = {
            "xT": xTr.astype(ml_dtypes.bfloat16),
            "wq": _shuffle_w(np.asarray(Wq, np.float32)[:, cols], c, wsc, wdt),
            "wk": _shuffle_w(np.asarray(Wk, np.float32)[:, cols], c, wsc, wdt),
            "wv": _shuffle_w(np.asarray(Wv, np.float32)[:, cols], c),
            "bq": np.ascontiguousarray(
                np.asarray(bq, np.float32)[cols].reshape(hk, P).T),
            "bk": np.ascontiguousarray(
                np.asarray(bk, np.float32)[cols].reshape(hk, P).T),
            "bv": np.asarray(bv, np.float32)[cols],
            "mask": masks,
        }
        if PROJ8:
            im["xT8"] = xTr.astype(f8)
        in_maps.append(im)
    return in_maps


def assemble(results, t, h):
    """Concat per-core [t, h/2] outputs back to [B, t, h]."""
    hh = h // 2
    out = np.empty((B, t, h), dtype=np.float32)
    for core in range(8):
        b, half = divmod(core, 2)
        out[b][:, half * hh:(half + 1) * hh] = results[core]["out"]
    return out


def kernel(x, Wq, bq, Wk, bk, Wv, bv):
    t, c, h = x.shape[1], x.shape[2], Wq.shape[1]
    runner = get_runner(t, c, h)
    results = runner(make_in_maps(x, Wq, bq, Wk, bk, Wv, bv))
    return assemble(results, t, h)

